# revision 1
# baseline (speedup 1.0000x reference)
"""Causal GQA self-attention (RoPE + qk-RMSNorm) Trainium2 Bass kernel.

Sharding over 8 NeuronCores: core = (b, g) with b = batch (2), g = kv-head
group (4). Each core computes qkv for its group's columns, RoPE + RMS norm,
causal attention for its 4 query heads against its 1 kv head, and a partial
c_proj (rows of w_proj owned by its heads). Host sums the 4 partials per
batch (row-parallel linear unshard).

Device layout notes:
  - qkv is computed in natural (T-partition) layout from PE-transposed x
    tiles; RoPE/RMS run in natural layout; q-hat/k-hat are then PE-transposed
    to (head-dim, T) layout so attention scores come out transposed
    (Tk on partitions, Tq free). Softmax needs no max subtraction (RMS-normed
    vectors bound |score| <= 8); denominators come from an appended ones
    column on v, and 1/denom is broadcast across partitions with a K=1
    ones-matmul. The (HD, Tq) y-transposed layout feeds c_proj's lhsT
    directly, so no further transposes are needed.
  - Matmuls run in bf16 (full PE rate, fp32 PSUM accumulation); RoPE, RMS
    and softmax arithmetic stay fp32. fp32r is rejected by this walrus
    build (setupSyncWait codegen failure on the LW struct).
"""

import numpy as np

B, T, C = 2, 2048, 1024
NH, NKV, HD = 16, 4, 64
NREP = NH // NKV          # 4 query heads per kv group
QD = NH * HD              # 1024
KVD = NKV * HD            # 256
LQ = NREP * HD            # 256 local q cols per core
NG = NREP + 1             # 5 rope/rms groups (4 q heads + 1 k head)
ROPE_BASE = 10000.0
EPS = 1e-6
NT = T // 128             # 16 row chunks
TQ = 512                  # q tile (free dim)
NTQ = T // TQ             # 4 q tiles
KC = C // 128             # 8 contraction chunks for qkv
N_CORES = 8

_CACHE: dict = {}


def _host_consts():
    pos = np.arange(T, dtype=np.float32)
    inv_freq = 1.0 / (ROPE_BASE ** (np.arange(0, HD, 2, dtype=np.float32) / HD))
    freqs = pos[:, None] * inv_freq[None, :]          # (T, 32)
    emb = np.repeat(freqs, 2, axis=-1)                # (T, 64) interleaved
    cos = np.cos(emb).astype(np.float32)
    sin = np.sin(emb).astype(np.float32)
    # chunked layout: [p, t*64+d] = cos[t*128+p, d]
    cos_c = np.ascontiguousarray(cos.reshape(NT, 128, HD).transpose(1, 0, 2))
    sin_c = np.ascontiguousarray(sin.reshape(NT, 128, HD).transpose(1, 0, 2))
    # diagonal masks: for delta in {0,128,256,384}: keep where delta+p <= f
    p = np.arange(128)[:, None]
    f = np.arange(TQ)[None, :]
    msk = np.stack(
        [(d + p <= f).astype(np.float32) for d in (0, 128, 256, 384)], axis=0
    )                                                  # (4,128,512)
    msk_c = np.ascontiguousarray(msk.transpose(1, 0, 2))  # (128,4,512)
    iden = np.eye(128, dtype=np.float32)
    return cos_c.reshape(128, NT * HD), sin_c.reshape(128, NT * HD), \
        msk_c.reshape(128, 4 * TQ), iden


def _legalize_waits(nc, mybir, max_waits=1):
    """Split multi-wait instructions: this walrus build's codegen rejects
    more than one sync wait per instruction ("Too many sync wait commands"),
    so hoist all but the last wait onto standalone same-engine
    InstEventSemaphore instructions placed immediately before."""
    n_split = 0
    for func in nc.m.functions:
        for bb in func.blocks:
            need = False
            for insn in bb.instructions:
                si = insn.sync_info
                if si is not None and len(si.on_wait) > max_waits:
                    need = True
                    break
            if not need:
                continue
            out = []
            for insn in bb.instructions:
                si = insn.sync_info
                if si is not None and len(si.on_wait) > max_waits:
                    extra = list(si.on_wait)[:-max_waits]
                    keep = list(si.on_wait)[-max_waits:]
                    for j, w in enumerate(extra):
                        out.append(mybir.InstEventSemaphore(
                            name=f"{insn.name}-hw{j}",
                            engine=insn.engine,
                            sync_info=mybir.SyncInfo(on_wait=[w], on_update=[]),
                        ))
                        n_split += 1
                    insn.sync_info = mybir.SyncInfo(
                        on_wait=keep, on_update=list(si.on_update))
                out.append(insn)
            bb.instructions = out
    return n_split


def _build_nc(legalize=True):
    import concourse.bass as bass
    import concourse.tile as tile
    import concourse.mybir as mybir
    from contextlib import ExitStack

    f32 = mybir.dt.float32
    bf16 = mybir.dt.bfloat16
    AF = mybir.ActivationFunctionType

    nc = bass.Bass()
    x_d = nc.dram_tensor("x", [T, C], bf16, kind="ExternalInput")
    wq_d = nc.dram_tensor("wq", [128, KC * 384], bf16, kind="ExternalInput")
    wp_d = nc.dram_tensor("wp", [64, 4 * C], bf16, kind="ExternalInput")
    cos_d = nc.dram_tensor("cos", [128, NT * HD], f32, kind="ExternalInput")
    sin_d = nc.dram_tensor("sin", [128, NT * HD], f32, kind="ExternalInput")
    msk_d = nc.dram_tensor("msk", [128, 4 * TQ], bf16, kind="ExternalInput")
    idn_d = nc.dram_tensor("idn", [128, 128], bf16, kind="ExternalInput")
    out_d = nc.dram_tensor("out", [T, C], f32, kind="ExternalOutput")

    with ExitStack() as ctx:
        tc = ctx.enter_context(tile.TileContext(nc))
        const = ctx.enter_context(tc.tile_pool(name="const", bufs=1))
        resid = ctx.enter_context(tc.tile_pool(name="resid", bufs=1))

        w_sb = const.tile([128, KC, 384], bf16)
        nc.sync.dma_start(w_sb[:].rearrange("p k n -> p (k n)"), wq_d[:])
        wp_sb = const.tile([64, 4, C], bf16)
        nc.sync.dma_start(wp_sb[:].rearrange("p k n -> p (k n)"), wp_d[:])
        cos_sb = const.tile([128, NT, HD], f32)
        nc.sync.dma_start(cos_sb[:].rearrange("p t d -> p (t d)"), cos_d[:])
        sin_sb = const.tile([128, NT, HD], f32)
        nc.sync.dma_start(sin_sb[:].rearrange("p t d -> p (t d)"), sin_d[:])
        msk_sb = const.tile([128, 4, TQ], bf16)
        nc.sync.dma_start(msk_sb[:].rearrange("p t d -> p (t d)"), msk_d[:])
        idn_sb = const.tile([128, 128], bf16)
        nc.sync.dma_start(idn_sb[:], idn_d[:])
        ones2_sb = const.tile([128, 128], bf16)
        nc.gpsimd.memset(ones2_sb[:], 1.0)
        eps_sb = const.tile([128, 1], f32)
        nc.gpsimd.memset(eps_sb[:], EPS)

        x_sb = resid.tile([128, NT, C], bf16)    # all of x, chunk-major
        nc.sync.dma_start(
            x_sb[:], x_d[:].rearrange("(t p) c -> p t c", p=128))
        qT_sb = resid.tile([128, 2, T], bf16)    # q-hat transposed
        kT_sb = resid.tile([128, T], bf16)       # k-hat transposed, duplicated
                                                 # on both partition halves
        va_sb = resid.tile([128, NT, HD + 1], bf16)   # [v | 1]
        yTe_sb = resid.tile([64, 2, T], bf16)     # yT for even heads (0, 2)
        yTo_sb = resid.tile([64, 2, T], bf16)     # yT for odd heads (1, 3)
        nc.gpsimd.memset(va_sb[:], 1.0)

        # ---------------- Phase 1: qkv + rope + rms + transposes ----------
        with tc.tile_pool(name="ph1", bufs=2) as ph1, \
             tc.tile_pool(name="pst", bufs=4, space="PSUM") as pst, \
             tc.tile_pool(name="psq", bufs=2, space="PSUM") as psq:
            for t in range(NT):
                xT = ph1.tile([128, KC, 128], bf16, tag="xT")
                for k in range(KC):
                    pt = pst.tile([128, 128], bf16, tag="pt")
                    nc.tensor.transpose(
                        pt[:], x_sb[:, t, k * 128:(k + 1) * 128], idn_sb[:])
                    nc.vector.tensor_copy(xT[:, k, :], pt[:])
                qkv = psq.tile([128, 384], f32, tag="qkv")
                for k in range(KC):
                    nc.tensor.matmul(
                        qkv[:], lhsT=xT[:, k, :], rhs=w_sb[:, k, :],
                        start=(k == 0), stop=(k == KC - 1))
                # RoPE over q+k region (128, 5, 64)
                qk = qkv[:, 0:NG * HD].rearrange("p (g d) -> p g d", g=NG)
                qk4 = qk.rearrange("p g (d two) -> p g d two", two=2)
                rot = ph1.tile([128, NG, HD], f32, tag="rot")
                rot4 = rot[:].rearrange("p g (d two) -> p g d two", two=2)
                nc.vector.tensor_scalar_mul(rot4[:, :, :, 0], qk4[:, :, :, 1], -1.0)
                nc.vector.tensor_copy(rot4[:, :, :, 1], qk4[:, :, :, 0])
                cosb = cos_sb[:, t, :].rearrange(
                    "p (g d) -> p g d", g=1).to_broadcast((128, NG, HD))
                sinb = sin_sb[:, t, :].rearrange(
                    "p (g d) -> p g d", g=1).to_broadcast((128, NG, HD))
                qkr = ph1.tile([128, NG, HD], f32, tag="qkr")
                nc.vector.tensor_mul(qkr[:], qk, cosb)
                nc.vector.tensor_mul(rot[:], rot[:], sinb)
                nc.vector.tensor_add(qkr[:], qkr[:], rot[:])
                # RMS norm per 64-group
                sq = ph1.tile([128, NG, HD], f32, tag="sq")
                nc.scalar.square(sq[:], qkr[:])
                ssum = ph1.tile([128, NG], f32, tag="ssum")
                nc.vector.tensor_reduce(
                    ssum[:], sq[:], axis=mybir.AxisListType.X,
                    op=mybir.AluOpType.add)
                srt = ph1.tile([128, NG], f32, tag="srt")
                nc.scalar.activation(srt[:], ssum[:], AF.Sqrt,
                                     bias=eps_sb[:], scale=1.0 / HD)
                rsc = ph1.tile([128, NG], f32, tag="rsc")
                nc.vector.reciprocal(rsc[:], srt[:])
                qkn = ph1.tile([128, NG, HD], bf16, tag="qkn")
                rscb = rsc[:].rearrange(
                    "p (g d) -> p g d", d=1).to_broadcast((128, NG, HD))
                nc.vector.tensor_mul(qkn[:], qkr[:], rscb)
                # v into v_aug (ones column survives from memset)
                nc.scalar.copy(va_sb[:, t, 0:HD], qkv[:, NG * HD:NG * HD + HD])
                # transpose q-hat (2 blocks) and k-hat (1 block)
                qkn_f = qkn[:].rearrange("p g d -> p (g d)")
                for qc in range(2):
                    pt = pst.tile([128, 128], bf16, tag="pt")
                    nc.tensor.transpose(
                        pt[:], qkn_f[:, qc * 128:(qc + 1) * 128], idn_sb[:])
                    nc.vector.tensor_copy(
                        qT_sb[:, qc, t * 128:(t + 1) * 128], pt[:])
                kk2 = ph1.tile([128, 2, HD], bf16, tag="kk2")
                nc.vector.tensor_copy(
                    kk2[:], qkn[:, NG - 1:NG, :].to_broadcast((128, 2, HD)))
                ptk = pst.tile([128, 128], bf16, tag="pt")
                nc.tensor.transpose(
                    ptk[:], kk2[:].rearrange("p a b -> p (a b)"), idn_sb[:])
                nc.vector.tensor_copy(
                    kT_sb[:, t * 128:(t + 1) * 128], ptk[:])

        # ---------------- Phase 2: causal attention -----------------------
        with tc.tile_pool(name="prb", bufs=4) as prb_p, \
             tc.tile_pool(name="pss", bufs=3, space="PSUM") as pss, \
             tc.tile_pool(name="psy", bufs=2, space="PSUM") as psy, \
             tc.tile_pool(name="psb", bufs=2, space="PSUM") as psb:
            for h in range(NREP):
                hp = (h % 2) * 64          # partition offset in qT/yT
                hc = h // 2                # column-chunk in qT/yT
                for qt in range(NTQ):
                    tq0 = qt * TQ
                    nblk = tq0 // 128 + TQ // 128
                    ps_y = psy.tile([128, TQ], f32, tag="ps_y")
                    dp = 64  # denom partition: [v|1] puts it at row 64
                    for tkb in range(nblk):
                        ps_s = pss.tile([128, TQ], f32, tag="ps_s")
                        nc.tensor.matmul(
                            ps_s[:],
                            lhsT=kT_sb[hp:hp + 64, tkb * 128:(tkb + 1) * 128],
                            rhs=qT_sb[hp:hp + 64, hc, tq0:tq0 + TQ],
                            start=True, stop=True)
                        probs = prb_p.tile([128, TQ], bf16, tag="probs")
                        nc.scalar.activation(probs[:], ps_s[:], AF.Exp,
                                             scale=0.125)
                        delta = tkb * 128 - tq0
                        if delta >= 0:
                            nc.vector.tensor_mul(
                                probs[:], probs[:], msk_sb[:, delta // 128, :])
                        nc.tensor.matmul(
                            ps_y[0:HD + 1, :],
                            lhsT=va_sb[:, tkb, :], rhs=probs[:],
                            start=(tkb == 0), stop=(tkb == nblk - 1))
                    rdt = prb_p.tile([128, TQ], f32, tag="rdt")
                    nc.vector.reciprocal(rdt[dp:dp + 1, :], ps_y[dp:dp + 1, :])
                    # hi/lo bf16 split so the broadcast keeps fp32 accuracy
                    rdh = prb_p.tile([128, TQ], bf16, tag="rdh")
                    nc.scalar.copy(rdh[dp:dp + 1, :], rdt[dp:dp + 1, :])
                    rdl = prb_p.tile([128, TQ], bf16, tag="rdl")
                    nc.vector.tensor_sub(rdl[dp:dp + 1, :], rdt[dp:dp + 1, :],
                                         rdh[dp:dp + 1, :])
                    ps_b = psb.tile([128, TQ], f32, tag="ps_b")
                    nc.tensor.matmul(
                        ps_b[:], lhsT=ones2_sb[dp:dp + 1, :],
                        rhs=rdh[dp:dp + 1, :], start=True, stop=False)
                    nc.tensor.matmul(
                        ps_b[:], lhsT=ones2_sb[dp:dp + 1, :],
                        rhs=rdl[dp:dp + 1, :], start=False, stop=True)
                    yraw = prb_p.tile([64, TQ], f32, tag="yraw")
                    nc.scalar.copy(yraw[:], ps_y[0:HD, :])
                    ydst = yTe_sb if hp == 0 else yTo_sb
                    nc.vector.tensor_mul(
                        ydst[0:HD, hc, tq0:tq0 + TQ],
                        yraw[:], ps_b[0:HD, :])

        # ---------------- Phase 3: c_proj partial --------------------------
        with tc.tile_pool(name="pso", bufs=4, space="PSUM") as pso, \
             tc.tile_pool(name="osb", bufs=4) as osb_p:
            for m in range(NT):
                for n in range(2):
                    ps_o = pso.tile([128, TQ], f32, tag="ps_o")
                    for h in range(NREP):
                        ysrc = yTe_sb if h % 2 == 0 else yTo_sb
                        nc.tensor.matmul(
                            ps_o[:],
                            lhsT=ysrc[0:HD, h // 2, m * 128:(m + 1) * 128],
                            rhs=wp_sb[0:HD, h, n * TQ:(n + 1) * TQ],
                            start=(h == 0), stop=(h == NREP - 1))
                    o_sb = osb_p.tile([128, TQ], f32, tag="o_sb")
                    nc.vector.tensor_copy(o_sb[:], ps_o[:])
                    nc.sync.dma_start(
                        out_d[m * 128:(m + 1) * 128, n * TQ:(n + 1) * TQ],
                        o_sb[:])
    if legalize:
        _legalize_waits(nc, mybir)
    return nc


def _get_nc(legalize=True):
    key = ("nc", legalize)
    if key not in _CACHE:
        _CACHE[key] = _build_nc(legalize)
    return _CACHE[key]


def make_in_maps(x, w_attn, w_proj):
    import ml_dtypes
    bf16 = ml_dtypes.bfloat16
    x = np.asarray(x, dtype=np.float32)
    w_attn = np.asarray(w_attn, dtype=np.float32)
    w_proj = np.asarray(w_proj, dtype=np.float32)
    cos_c, sin_c, msk_c, iden = _host_consts()
    msk_c = msk_c.astype(bf16)
    iden = iden.astype(bf16)
    in_maps = []
    for core in range(N_CORES):
        b, g = divmod(core, NKV)
        wq = w_attn[:, g * LQ:(g + 1) * LQ]
        wk = w_attn[:, QD + g * HD:QD + (g + 1) * HD]
        wv = w_attn[:, QD + KVD + g * HD:QD + KVD + (g + 1) * HD]
        wqkv = np.concatenate([wq, wk, wv], axis=1)          # (1024, 384)
        wq_c = np.ascontiguousarray(
            wqkv.reshape(KC, 128, 384).transpose(1, 0, 2)
        ).reshape(128, KC * 384).astype(bf16)
        wp_c = np.ascontiguousarray(
            w_proj[g * LQ:(g + 1) * LQ].reshape(4, 64, C).transpose(1, 0, 2)
        ).reshape(64, 4 * C).astype(bf16)
        in_maps.append({
            "x": np.ascontiguousarray(x[b]).astype(bf16),
            "wq": wq_c, "wp": wp_c,
            "cos": cos_c, "sin": sin_c, "msk": msk_c, "idn": iden,
        })
    return in_maps


def run_spmd(x, w_attn, w_proj, trace=False):
    from concourse.bass_utils import run_bass_kernel_spmd
    nc = _get_nc()
    in_maps = make_in_maps(x, w_attn, w_proj)
    br = run_bass_kernel_spmd(nc, in_maps, list(range(N_CORES)), trace=trace)
    out = np.empty((B, T, C), dtype=np.float32)
    for b in range(B):
        acc = br.results[NKV * b]["out"].astype(np.float32)
        for g in range(1, NKV):
            acc = acc + br.results[NKV * b + g]["out"]
        out[b] = acc
    return out, br


def kernel(x, w_attn, w_proj):
    out, _ = run_spmd(x, w_attn, w_proj, trace=False)
    return out



# revision 2
# speedup vs baseline: 1.0311x; 1.0311x over previous
"""Causal GQA self-attention (RoPE + qk-RMSNorm) Trainium2 Bass kernel.

Sharding over 8 NeuronCores: core = (b, g) with b = batch (2), g = kv-head
group (4). Each core computes qkv for its group's columns, RoPE + RMS norm,
causal attention for its 4 query heads against its 1 kv head, and a partial
c_proj (rows of w_proj owned by its heads). Host sums the 4 partials per
batch (row-parallel linear unshard).

Device layout notes:
  - qkv is computed in natural (T-partition) layout from PE-transposed x
    tiles; RoPE/RMS run in natural layout; q-hat/k-hat are then PE-transposed
    to (head-dim, T) layout so attention scores come out transposed
    (Tk on partitions, Tq free). Softmax needs no max subtraction (RMS-normed
    vectors bound |score| <= 8); denominators come from an appended ones
    column on v, and 1/denom is broadcast across partitions with a K=1
    ones-matmul. The (HD, Tq) y-transposed layout feeds c_proj's lhsT
    directly, so no further transposes are needed.
  - Matmuls run in bf16 (full PE rate, fp32 PSUM accumulation); RoPE, RMS
    and softmax arithmetic stay fp32. fp32r is rejected by this walrus
    build (setupSyncWait codegen failure on the LW struct).
"""

import numpy as np

B, T, C = 2, 2048, 1024
NH, NKV, HD = 16, 4, 64
NREP = NH // NKV          # 4 query heads per kv group
QD = NH * HD              # 1024
KVD = NKV * HD            # 256
LQ = NREP * HD            # 256 local q cols per core
NG = NREP + 1             # 5 rope/rms groups (4 q heads + 1 k head)
ROPE_BASE = 10000.0
EPS = 1e-6
NT = T // 128             # 16 row chunks
TQ = 512                  # q tile (free dim)
NTQ = T // TQ             # 4 q tiles
KC = C // 128             # 8 contraction chunks for qkv
N_CORES = 8

_CACHE: dict = {}


def _host_consts():
    pos = np.arange(T, dtype=np.float32)
    inv_freq = 1.0 / (ROPE_BASE ** (np.arange(0, HD, 2, dtype=np.float32) / HD))
    freqs = pos[:, None] * inv_freq[None, :]          # (T, 32)
    emb = np.repeat(freqs, 2, axis=-1)                # (T, 64) interleaved
    cos = np.cos(emb).astype(np.float32)
    sin = np.sin(emb).astype(np.float32)
    # chunked layout: [p, t*64+d] = cos[t*128+p, d]
    cos_c = np.ascontiguousarray(cos.reshape(NT, 128, HD).transpose(1, 0, 2))
    sin_c = np.ascontiguousarray(sin.reshape(NT, 128, HD).transpose(1, 0, 2))
    # diagonal masks: for delta in {0,128,256,384}: keep where delta+p <= f
    p = np.arange(128)[:, None]
    f = np.arange(TQ)[None, :]
    msk = np.stack(
        [(d + p <= f).astype(np.float32) for d in (0, 128, 256, 384)], axis=0
    )                                                  # (4,128,512)
    msk_c = np.ascontiguousarray(msk.transpose(1, 0, 2))  # (128,4,512)
    iden = np.eye(128, dtype=np.float32)
    return cos_c.reshape(128, NT * HD), sin_c.reshape(128, NT * HD), \
        msk_c.reshape(128, 4 * TQ), iden


def _legalize_waits(nc, mybir, max_waits=1):
    """Split multi-wait instructions: this walrus build's codegen rejects
    more than one sync wait per instruction ("Too many sync wait commands"),
    so hoist all but the last wait onto standalone same-engine
    InstEventSemaphore instructions placed immediately before."""
    n_split = 0
    for func in nc.m.functions:
        for bb in func.blocks:
            need = False
            for insn in bb.instructions:
                si = insn.sync_info
                if si is not None and len(si.on_wait) > max_waits:
                    need = True
                    break
            if not need:
                continue
            out = []
            for insn in bb.instructions:
                si = insn.sync_info
                if si is not None and len(si.on_wait) > max_waits:
                    extra = list(si.on_wait)[:-max_waits]
                    keep = list(si.on_wait)[-max_waits:]
                    for j, w in enumerate(extra):
                        out.append(mybir.InstEventSemaphore(
                            name=f"{insn.name}-hw{j}",
                            engine=insn.engine,
                            sync_info=mybir.SyncInfo(on_wait=[w], on_update=[]),
                        ))
                        n_split += 1
                    insn.sync_info = mybir.SyncInfo(
                        on_wait=keep, on_update=list(si.on_update))
                out.append(insn)
            bb.instructions = out
    return n_split


def _build_nc(legalize=True, loop_iters=1):
    import concourse.bass as bass
    import concourse.tile as tile
    import concourse.mybir as mybir
    from contextlib import ExitStack, nullcontext

    f32 = mybir.dt.float32
    bf16 = mybir.dt.bfloat16
    AF = mybir.ActivationFunctionType

    nc = bass.Bass()
    x_d = nc.dram_tensor("x", [T, C], bf16, kind="ExternalInput")
    wq_d = nc.dram_tensor("wq", [128, KC * 384], bf16, kind="ExternalInput")
    wp_d = nc.dram_tensor("wp", [64, 4 * C], bf16, kind="ExternalInput")
    cos_d = nc.dram_tensor("cos", [128, NT * HD], f32, kind="ExternalInput")
    sin_d = nc.dram_tensor("sin", [128, NT * HD], f32, kind="ExternalInput")
    msk_d = nc.dram_tensor("msk", [128, 4 * TQ], bf16, kind="ExternalInput")
    idn_d = nc.dram_tensor("idn", [128, 128], bf16, kind="ExternalInput")
    out_d = nc.dram_tensor("out", [T, C], f32, kind="ExternalOutput")

    with ExitStack() as ctx:
        tc = ctx.enter_context(tile.TileContext(nc))
        const = ctx.enter_context(tc.tile_pool(name="const", bufs=1))
        resid = ctx.enter_context(tc.tile_pool(name="resid", bufs=1))

        w_sb = const.tile([128, KC, 384], bf16)
        nc.sync.dma_start(w_sb[:].rearrange("p k n -> p (k n)"), wq_d[:])
        wp_sb = const.tile([64, 4, C], bf16)
        nc.sync.dma_start(wp_sb[:].rearrange("p k n -> p (k n)"), wp_d[:])
        cos_sb = const.tile([128, NT, HD], f32)
        nc.sync.dma_start(cos_sb[:].rearrange("p t d -> p (t d)"), cos_d[:])
        sin_sb = const.tile([128, NT, HD], f32)
        nc.sync.dma_start(sin_sb[:].rearrange("p t d -> p (t d)"), sin_d[:])
        msk_sb = const.tile([128, 4, TQ], bf16)
        nc.sync.dma_start(msk_sb[:].rearrange("p t d -> p (t d)"), msk_d[:])
        idn_sb = const.tile([128, 128], bf16)
        nc.sync.dma_start(idn_sb[:], idn_d[:])
        ones2_sb = const.tile([128, 128], bf16)
        nc.gpsimd.memset(ones2_sb[:], 1.0)
        eps_sb = const.tile([128, 1], f32)
        nc.gpsimd.memset(eps_sb[:], EPS)

        x_sb = resid.tile([128, NT, C], bf16)    # all of x, chunk-major
        nc.sync.dma_start(
            x_sb[:], x_d[:].rearrange("(t p) c -> p t c", p=128))
        qT_sb = resid.tile([128, 2, T], bf16)    # q-hat transposed
        kT_sb = resid.tile([128, T], bf16)       # k-hat transposed, duplicated
                                                 # on both partition halves
        va_sb = resid.tile([128, NT, HD + 1], bf16)   # [v | 1]
        yTe_sb = resid.tile([64, 2, T], bf16)     # yT for even heads (0, 2)
        yTo_sb = resid.tile([64, 2, T], bf16)     # yT for odd heads (1, 3)
        nc.gpsimd.memset(va_sb[:], 1.0)

        # ---------------- Phase 1: qkv + rope + rms + transposes ----------
        with tc.tile_pool(name="ph1", bufs=2) as ph1, \
             tc.tile_pool(name="pst", bufs=4, space="PSUM") as pst, \
             tc.tile_pool(name="psq", bufs=2, space="PSUM") as psq:
            for t in range(NT):
                xT = ph1.tile([128, KC, 128], bf16, tag="xT")
                for k in range(KC):
                    pt = pst.tile([128, 128], bf16, tag="pt")
                    nc.tensor.transpose(
                        pt[:], x_sb[:, t, k * 128:(k + 1) * 128], idn_sb[:])
                    nc.vector.tensor_copy(xT[:, k, :], pt[:])
                qkv = psq.tile([128, 384], f32, tag="qkv")
                for k in range(KC):
                    nc.tensor.matmul(
                        qkv[:], lhsT=xT[:, k, :], rhs=w_sb[:, k, :],
                        start=(k == 0), stop=(k == KC - 1))
                # RoPE over q+k region (128, 5, 64)
                qk = qkv[:, 0:NG * HD].rearrange("p (g d) -> p g d", g=NG)
                qk4 = qk.rearrange("p g (d two) -> p g d two", two=2)
                rot = ph1.tile([128, NG, HD], f32, tag="rot")
                rot4 = rot[:].rearrange("p g (d two) -> p g d two", two=2)
                nc.vector.tensor_scalar_mul(rot4[:, :, :, 0], qk4[:, :, :, 1], -1.0)
                nc.vector.tensor_copy(rot4[:, :, :, 1], qk4[:, :, :, 0])
                cosb = cos_sb[:, t, :].rearrange(
                    "p (g d) -> p g d", g=1).to_broadcast((128, NG, HD))
                sinb = sin_sb[:, t, :].rearrange(
                    "p (g d) -> p g d", g=1).to_broadcast((128, NG, HD))
                qkr = ph1.tile([128, NG, HD], f32, tag="qkr")
                nc.vector.tensor_mul(qkr[:], qk, cosb)
                nc.vector.tensor_mul(rot[:], rot[:], sinb)
                nc.vector.tensor_add(qkr[:], qkr[:], rot[:])
                # RMS norm per 64-group
                sq = ph1.tile([128, NG, HD], f32, tag="sq")
                nc.scalar.square(sq[:], qkr[:])
                ssum = ph1.tile([128, NG], f32, tag="ssum")
                nc.vector.tensor_reduce(
                    ssum[:], sq[:], axis=mybir.AxisListType.X,
                    op=mybir.AluOpType.add)
                srt = ph1.tile([128, NG], f32, tag="srt")
                nc.scalar.activation(srt[:], ssum[:], AF.Sqrt,
                                     bias=eps_sb[:], scale=1.0 / HD)
                rsc = ph1.tile([128, NG], f32, tag="rsc")
                nc.vector.reciprocal(rsc[:], srt[:])
                qkn = ph1.tile([128, NG, HD], bf16, tag="qkn")
                rscb = rsc[:].rearrange(
                    "p (g d) -> p g d", d=1).to_broadcast((128, NG, HD))
                nc.vector.tensor_mul(qkn[:], qkr[:], rscb)
                # v into v_aug (ones column survives from memset)
                nc.scalar.copy(va_sb[:, t, 0:HD], qkv[:, NG * HD:NG * HD + HD])
                # transpose q-hat (2 blocks) and k-hat (1 block)
                qkn_f = qkn[:].rearrange("p g d -> p (g d)")
                for qc in range(2):
                    pt = pst.tile([128, 128], bf16, tag="pt")
                    nc.tensor.transpose(
                        pt[:], qkn_f[:, qc * 128:(qc + 1) * 128], idn_sb[:])
                    nc.vector.tensor_copy(
                        qT_sb[:, qc, t * 128:(t + 1) * 128], pt[:])
                kk2 = ph1.tile([128, 2, HD], bf16, tag="kk2")
                nc.vector.tensor_copy(
                    kk2[:], qkn[:, NG - 1:NG, :].to_broadcast((128, 2, HD)))
                ptk = pst.tile([128, 128], bf16, tag="pt")
                nc.tensor.transpose(
                    ptk[:], kk2[:].rearrange("p a b -> p (a b)"), idn_sb[:])
                nc.vector.tensor_copy(
                    kT_sb[:, t * 128:(t + 1) * 128], ptk[:])

        # ---------------- Phase 2: causal attention -----------------------
        with tc.tile_pool(name="prb", bufs=4) as prb_p, \
             tc.tile_pool(name="pss", bufs=3, space="PSUM") as pss, \
             tc.tile_pool(name="psy", bufs=2, space="PSUM") as psy, \
             tc.tile_pool(name="psb", bufs=2, space="PSUM") as psb:
            for h in range(NREP):
                hp = (h % 2) * 64          # partition offset in qT/yT
                hc = h // 2                # column-chunk in qT/yT
                for qt in range(NTQ):
                    tq0 = qt * TQ
                    nblk = tq0 // 128 + TQ // 128
                    ps_y = psy.tile([128, TQ], f32, tag="ps_y")
                    dp = 64  # denom partition: [v|1] puts it at row 64
                    for tkb in range(nblk):
                        ps_s = pss.tile([128, TQ], f32, tag="ps_s")
                        nc.tensor.matmul(
                            ps_s[:],
                            lhsT=kT_sb[hp:hp + 64, tkb * 128:(tkb + 1) * 128],
                            rhs=qT_sb[hp:hp + 64, hc, tq0:tq0 + TQ],
                            start=True, stop=True)
                        probs = prb_p.tile([128, TQ], bf16, tag="probs")
                        nc.scalar.activation(probs[:], ps_s[:], AF.Exp,
                                             scale=0.125)
                        delta = tkb * 128 - tq0
                        if delta >= 0:
                            nc.vector.tensor_mul(
                                probs[:], probs[:], msk_sb[:, delta // 128, :])
                        nc.tensor.matmul(
                            ps_y[0:HD + 1, :],
                            lhsT=va_sb[:, tkb, :], rhs=probs[:],
                            start=(tkb == 0), stop=(tkb == nblk - 1))
                    rdt = prb_p.tile([128, TQ], f32, tag="rdt")
                    nc.vector.reciprocal(rdt[dp:dp + 1, :], ps_y[dp:dp + 1, :])
                    # hi/lo bf16 split so the broadcast keeps fp32 accuracy
                    rdh = prb_p.tile([128, TQ], bf16, tag="rdh")
                    nc.scalar.copy(rdh[dp:dp + 1, :], rdt[dp:dp + 1, :])
                    rdl = prb_p.tile([128, TQ], bf16, tag="rdl")
                    nc.vector.tensor_sub(rdl[dp:dp + 1, :], rdt[dp:dp + 1, :],
                                         rdh[dp:dp + 1, :])
                    ps_b = psb.tile([128, TQ], f32, tag="ps_b")
                    nc.tensor.matmul(
                        ps_b[:], lhsT=ones2_sb[dp:dp + 1, :],
                        rhs=rdh[dp:dp + 1, :], start=True, stop=False)
                    nc.tensor.matmul(
                        ps_b[:], lhsT=ones2_sb[dp:dp + 1, :],
                        rhs=rdl[dp:dp + 1, :], start=False, stop=True)
                    yraw = prb_p.tile([64, TQ], f32, tag="yraw")
                    nc.scalar.copy(yraw[:], ps_y[0:HD, :])
                    ydst = yTe_sb if hp == 0 else yTo_sb
                    nc.vector.tensor_mul(
                        ydst[0:HD, hc, tq0:tq0 + TQ],
                        yraw[:], ps_b[0:HD, :])

        # ---------------- Phase 3: c_proj partial --------------------------
        with tc.tile_pool(name="pso", bufs=4, space="PSUM") as pso, \
             tc.tile_pool(name="osb", bufs=4) as osb_p:
            for m in range(NT):
                for n in range(2):
                    ps_o = pso.tile([128, TQ], f32, tag="ps_o")
                    for h in range(NREP):
                        ysrc = yTe_sb if h % 2 == 0 else yTo_sb
                        nc.tensor.matmul(
                            ps_o[:],
                            lhsT=ysrc[0:HD, h // 2, m * 128:(m + 1) * 128],
                            rhs=wp_sb[0:HD, h, n * TQ:(n + 1) * TQ],
                            start=(h == 0), stop=(h == NREP - 1))
                    o_sb = osb_p.tile([128, TQ], f32, tag="o_sb")
                    nc.vector.tensor_copy(o_sb[:], ps_o[:])
                    nc.sync.dma_start(
                        out_d[m * 128:(m + 1) * 128, n * TQ:(n + 1) * TQ],
                        o_sb[:])
    if legalize:
        _legalize_waits(nc, mybir)
    return nc


def _get_nc(legalize=True):
    key = ("nc", legalize)
    if key not in _CACHE:
        _CACHE[key] = _build_nc(legalize)
    return _CACHE[key]


def make_in_maps(x, w_attn, w_proj):
    import ml_dtypes
    bf16 = ml_dtypes.bfloat16
    x = np.asarray(x, dtype=np.float32)
    w_attn = np.asarray(w_attn, dtype=np.float32)
    w_proj = np.asarray(w_proj, dtype=np.float32)
    cos_c, sin_c, msk_c, iden = _host_consts()
    msk_c = msk_c.astype(bf16)
    iden = iden.astype(bf16)
    in_maps = []
    for core in range(N_CORES):
        b, g = divmod(core, NKV)
        wq = w_attn[:, g * LQ:(g + 1) * LQ]
        wk = w_attn[:, QD + g * HD:QD + (g + 1) * HD]
        wv = w_attn[:, QD + KVD + g * HD:QD + KVD + (g + 1) * HD]
        wqkv = np.concatenate([wq, wk, wv], axis=1)          # (1024, 384)
        wq_c = np.ascontiguousarray(
            wqkv.reshape(KC, 128, 384).transpose(1, 0, 2)
        ).reshape(128, KC * 384).astype(bf16)
        wp_c = np.ascontiguousarray(
            w_proj[g * LQ:(g + 1) * LQ].reshape(4, 64, C).transpose(1, 0, 2)
        ).reshape(64, 4 * C).astype(bf16)
        in_maps.append({
            "x": np.ascontiguousarray(x[b]).astype(bf16),
            "wq": wq_c, "wp": wp_c,
            "cos": cos_c, "sin": sin_c, "msk": msk_c, "idn": iden,
        })
    return in_maps


def run_spmd(x, w_attn, w_proj, trace=False):
    from concourse.bass_utils import run_bass_kernel_spmd
    nc = _get_nc()
    in_maps = make_in_maps(x, w_attn, w_proj)
    br = run_bass_kernel_spmd(nc, in_maps, list(range(N_CORES)), trace=trace)
    out = np.empty((B, T, C), dtype=np.float32)
    for b in range(B):
        acc = br.results[NKV * b]["out"].astype(np.float32)
        for g in range(1, NKV):
            acc = acc + br.results[NKV * b + g]["out"]
        out[b] = acc
    return out, br


def kernel(x, w_attn, w_proj):
    out, _ = run_spmd(x, w_attn, w_proj, trace=False)
    return out

